# revision 3
# baseline (speedup 1.0000x reference)
"""Trainium2 Bass kernel for nn_CustomParameterTransform (scatter_memory).

Reference semantics: coord_v [256, 30] holds 10 (x, y, mass) triplets per
sample. Each triplet maps to integer grid indices (x_i, y_i, m_i); a one-hot
volume z [B, 16, 128, 128] is scattered (z[b, m, y, x] = 1) and the output is
concat(1-z, z) over the channel axis -> [256, 32, 128, 128] f32 (512 MB).

Strategy (8 NeuronCores, batch-sharded, no cross-core comm): single-SWDGE-queue
design. Per core the 64 MB output slab is mostly constant (ones-half / zeros-
half per sample); the 640 scatter points are fixed up with indirect DMAs.

All fills AND all scatters ride the one gpsimd SWDGE queue (qPoolDynamic).
Each SDMA engine drains its ring slot in FIFO order, and descriptors are
assigned to engines by SOURCE partition. The host places every scatter point's
offset on the offset ROW equal to the source partition of the fill chunk that
covers it, so the scatter descriptor lands on the same engine AFTER that fill's
chunk descriptor — write order is guaranteed by the per-engine ring FIFO with
NO fill->scatter semaphore edges. Two things make this hold:
  - nosync ordering edges chain every gpsimd DMA to the previous one, pinning
    the Tile scheduler to program order (emission order == ring order);
  - each scatter column's narrow out AP gets a distinct dep_tracking_offset so
    Tile doesn't WAW-chain the columns (that chain serialized at ~8 us/column).
The kernel tail is then just the ring drain + light sem clear: the scatter
descriptors sit at the very end of each engine's ring and complete with it.
"""

import numpy as np

B = 256
NSRC = 10
NMC = 16
L = 128
NCORES = 8
BL = B // NCORES          # 32 samples per core
PLANE = L * L             # 16384
HALF = NMC * PLANE        # 262144 elements per half-slab
SLAB = 2 * HALF           # 524288 elements per sample
OUT_ELEMS = BL * SLAB     # 16777216 per core (64 MB)

MINI = 131072             # elements per mini fill (512 KB)
CHUNK = 4096              # elements per partition chunk of a combo fill

_CACHE = {}


def _build_nc(K):
    import concourse.bass as bass
    import concourse.tile as tile
    from concourse import bacc, mybir
    from concourse.tile_rust import add_dep_helper

    import types as _types
    from concourse.vector_clock import ScopedClock

    # The const-AP registration in Bass.__init__ ends with an all-engine
    # barrier (~1.5 us of event-sem chaining at the head of every
    # execution). This kernel never touches const_aps -- memset packs its
    # immediate and the DMAs don't use them -- so elide the barrier for
    # the duration of construction.
    _orig_barrier = bass.Bass.all_engine_barrier
    bass.Bass.all_engine_barrier = lambda self, **kw: None
    try:
        nc = bacc.Bacc("TRN2", target_bir_lowering=False, debug=False,
                       num_devices=NCORES)
    finally:
        bass.Bass.all_engine_barrier = _orig_barrier

    def _light_drain_and_barrier(self, tick_clock, wait_clock):
        """Replaces TileContext._drain_and_barrier for this kernel. The
        stock epilogue is drain + two all-engine EVSEM butterfly barriers
        around the sem clear (~9 us after event lowering). Requirements at
        kernel end are: (1) all DMA completions observed, (2) sems cleared
        for NEFF re-execution, (3) the clear happens after every engine's
        last sem use. (1) is the sync drain's global-clock waits; (3) is a
        counting-sem join (sync arrives only after the drain, so join>=4
        implies all DMA done); (2) is the ranged clear. The second barrier
        is unnecessary: a re-execution cannot start until every engine --
        including the clearing gpsimd -- has ended."""
        nc_ = self.nc
        drain_inst = nc_.sync.drain()
        wait_clock.add_sem_waits(
            drain_inst.ins, ScopedClock({None: tick_clock.global_clock}))
        join = nc_.alloc_semaphore("tail_join")
        for eng in nc_.engines.values():
            if eng is not nc_.gpsimd:
                eng.sem_inc(join, 1)
        n_other = len(nc_.engines) - 1
        nc_.gpsimd.wait_ge(join, n_other)
        popped = nc_._tile_sem_poison_stack.pop()
        assert popped is self._sem_poison
        sems = list(self.sems.allocated().values())
        nc_.clear_and_free_semaphores(sems + [join])

    offs = nc.dram_tensor("offs", [128, K], mybir.dt.int32,
                          kind="ExternalInput").ap()
    vals = nc.dram_tensor("vals", [128, K], mybir.dt.float32,
                          kind="ExternalInput").ap()
    out = nc.dram_tensor("out", [OUT_ELEMS], mybir.dt.float32,
                         kind="ExternalOutput").ap()

    with tile.TileContext(nc) as tc:
        tc._drain_and_barrier = _types.MethodType(_light_drain_and_barrier, tc)
        with tc.tile_pool(name="src", bufs=1) as src_pool, \
             tc.tile_pool(name="small", bufs=1) as small_pool:
            ring = []   # gpsimd DMA instructions, in required ring order

            def chain(inst):
                if ring:
                    add_dep_helper(inst.ins, ring[-1].ins, sync=False,
                                   reason="SWDGE ring order")
                ring.append(inst)
                return inst

            # Input tables first: the queue is empty, the loads are tiny,
            # and the scatter emissions (which read offs_t on the Q7) only
            # run ~40 us in.
            offs_t = small_pool.tile([128, K], mybir.dt.int32)
            vals_t = small_pool.tile([128, K], mybir.dt.float32)
            chain(nc.gpsimd.dma_start(offs_t[:, :], offs[:, :]))
            chain(nc.gpsimd.dma_start(vals_t[:, :], vals[:, :]))

            # Constant source tiles, all memset on the vector engine so the
            # gpsimd Q7 stays free for descriptor emission. Memset cost
            # scales with free-length per lane, so the combo stripes
            # ([64, 4096] each) cost the same as a full-width memset.
            ones_mini = src_pool.tile([128, 1024], mybir.dt.float32)
            zeros_mini = src_pool.tile([128, 1024], mybir.dt.float32)
            nc.vector.memset(ones_mini[:, :], 1.0)
            nc.vector.memset(zeros_mini[:, :], 0.0)
            combo = src_pool.tile([128, CHUNK], mybir.dt.float32)
            nc.vector.memset(combo[0:64, :], 1.0)
            nc.vector.memset(combo[64:128, :], 0.0)

            # Slabs 0-1 from the minis (ready first; bridges until combo's
            # memsets land). Chunk row = (e % MINI)//1024.
            for s in (0, 1):
                for k in (0, 1):
                    lo = s * SLAB + k * MINI
                    chain(nc.gpsimd.dma_start(out[lo:lo + MINI],
                                              ones_mini[:, :]))
                for k in (0, 1):
                    lo = s * SLAB + HALF + k * MINI
                    chain(nc.gpsimd.dma_start(out[lo:lo + MINI],
                                              zeros_mini[:, :]))

            # Slabs 2-31 as 30 2MB fills from combo (partition p covers the
            # contiguous chunk [s*SLAB + p*4096, +4096): ones iff p < 64).
            for s in range(2, BL):
                chain(nc.gpsimd.dma_start(out[s * SLAB:(s + 1) * SLAB],
                                          combo[:, :]))

            # Scatter columns. Offsets sit on the row equal to the source
            # partition of the covering fill chunk, so each scatter
            # descriptor is served by the same SDMA engine as its fill
            # chunk and the ring FIFO orders them. The out AP is narrow
            # (offset 0, required by the indirect path) but carries a
            # per-column dep_tracking_offset so Tile neither serializes
            # the columns against each other nor against every fill.
            out2d = out[0:1].unsqueeze(1)
            for j in range(K):
                oap = bass.AP(tensor=out2d.tensor, offset=0, ap=out2d.ap,
                              dep_tracking_offset=j)
                chain(nc.gpsimd.indirect_dma_start(
                    out=oap,
                    out_offset=bass.IndirectOffsetOnAxis(
                        ap=offs_t[:, j:j + 1], axis=0),
                    in_=vals_t[:, j:j + 1],
                    in_offset=None,
                ))

    nc.compile()
    return nc


def _compute_indices(coord_v, lows, highs, nmc, L_):
    """Replicates reference.py lines exactly (same jax ops on the default
    device) so the floor/log10 bin boundaries match bit-for-bit."""
    import jax.numpy as jnp

    cv = jnp.asarray(np.asarray(coord_v, dtype=np.float32))
    n = cv.shape[1] // 3
    v10 = cv.at[:, 2::3].set(jnp.log10(cv[:, 2::3]))
    lo = jnp.tile(jnp.asarray(np.asarray(lows, dtype=np.float32)), n)
    hi = jnp.tile(jnp.asarray(np.asarray(highs, dtype=np.float32)), n)
    coord_grid = (v10 - lo) / (hi - lo)
    tr = coord_grid.reshape(-1, 3)
    x_i = jnp.floor(tr[:, 0] * L_).astype(jnp.int32)
    y_i = jnp.floor(tr[:, 1] * L_).astype(jnp.int32)
    m_i = jnp.floor(tr[:, 2] * nmc).astype(jnp.int32)
    return (np.asarray(x_i), np.asarray(y_i), np.asarray(m_i))


def _row_of(E):
    """Offset row (== source partition of the covering fill chunk) for a
    flat element offset E in the per-core out tensor."""
    if E < 2 * SLAB:   # mini fills: 1024-elem chunks
        return (E % MINI) // 1024
    return (E % SLAB) // CHUNK   # combo fills: 4096-elem chunks


def _prepare_in_maps(coord_v, lows, highs, nmc, L):
    nmc = int(nmc)
    L_ = int(L)
    x_i, y_i, m_i = _compute_indices(coord_v, lows, highs, nmc, L_)
    n_batch = coord_v.shape[0]
    n = coord_v.shape[1] // 3
    b_i = np.repeat(np.arange(n_batch, dtype=np.int64), n)

    flat_ones = ((b_i % BL) * SLAB + m_i.astype(np.int64) * PLANE
                 + y_i.astype(np.int64) * L_ + x_i.astype(np.int64))
    flat_z = flat_ones + HALF

    pts_per_core = BL * n  # 320
    per_core = []
    K = 1
    for c in range(NCORES):
        sel = slice(c * pts_per_core, (c + 1) * pts_per_core)
        # (offset, value) pairs; ones-half points write 0.0, z-half 1.0.
        pts = ([(int(e), 0.0) for e in flat_ones[sel]]
               + [(int(e), 1.0) for e in flat_z[sel]])
        rows = {}
        for e, v in pts:
            rows.setdefault(_row_of(e), []).append((e, v))
        per_core.append((pts, rows))
        K = max(K, max(len(l) for l in rows.values()))

    in_maps = []
    for c in range(NCORES):
        pts, rows = per_core[c]
        used = set(e for e, _ in pts)
        offs_np = np.zeros((128, K), dtype=np.int32)
        vals_np = np.zeros((128, K), dtype=np.float32)
        for r in range(128):
            lst = rows.get(r, [])
            if len(lst) < K:
                # Idempotent dummy: an element of slab 2 on this chunk row
                # (the combo fill writes 1.0 iff r < 64 there), avoiding
                # real point addresses.
                d = 2 * SLAB + r * CHUNK + 7
                while d in used:
                    d += 1
                v = 1.0 if r < 64 else 0.0
                lst = lst + [(d, v)] * (K - len(lst))
            for j, (e, v) in enumerate(lst):
                offs_np[r, j] = e
                vals_np[r, j] = v
        in_maps.append({"offs": offs_np, "vals": vals_np})
    return K, in_maps


def _run(K, in_maps, **kwargs):
    if _CACHE.get("K") != K:
        _CACHE["nc"] = _build_nc(K)
        _CACHE["K"] = K
    nc = _CACHE["nc"]
    from concourse.bass_utils import run_bass_kernel_spmd
    return run_bass_kernel_spmd(nc, in_maps, core_ids=list(range(NCORES)),
                                **kwargs)


def kernel(coord_v, lows, highs, nmc, L):
    nmc = int(nmc)
    L_ = int(L)
    assert nmc == NMC and L_ == globals()["L"], (nmc, L_)

    K, in_maps = _prepare_in_maps(coord_v, lows, highs, nmc, L_)
    res = _run(K, in_maps)
    parts = [res.results[c]["out"].reshape(BL, 2 * NMC, L_, L_)
             for c in range(NCORES)]
    return np.concatenate(parts, axis=0)
